# revision 32
# baseline (speedup 1.0000x reference)
"""Trainium2 Bass kernel for DUPN-style LSTM + windowed-softmax attention pooling.

Math (per batch element b):
  LSTM over T=128 steps (torch gate order), hidden H=512, input D=256.
  a[t] = sigmoid(x[t]·u1 + h[t]·u2), u1 = (v1@A1)^T, u2 = (v1@A2)^T  (folded)
  out[b,k,:] = softmax-pooled sum of h[t] over window t <= t_k, for 4 slots.

Sharding: data-parallel over batch, 32 per core x 8 cores, weights replicated.

Per-core schedule (v3):
  - Preamble precomputes xw = x@W_ih^T + bias for the WHOLE sequence as one
    continuous PE stream (full p-state, no loop contention), stored bf16
    (128KB/partition). a1 = x·u1 rides along per chunk.
  - Recurrent loop per step: 8 fp8-DoubleRow whh matmuls (K=256 per pass)
    accumulate onto per-gate PSUM tiles seeded by identity injects from the
    bf16 xw; per-gate [32,512] activations (order g,f,i,o) overlap the whh
    block; bf16 DVE c-chain; h via PE transposes of sig(o) and c + tanh on
    the 128-partition cT; DVE writes both the fp8 hT (next step's matmul
    stationary) and the bf16 hsT history directly.
  - a2 = u2·h_t for all (b,t) is one batched matmul pass post-loop.
  - Post-loop: windowed softmax with host-built masks, pooling via per-b
    [4,T]@[T,H] bf16 matmuls on PE-transposed hs.
  - Numerics: x-side bf16, W_hh fp8e4 (both operands), gates/c bf16,
    softmax in fp32. Validated ~7.5e-3 rel err vs fp32 reference.
"""
import sys

if "/opt/trn_rl_repo" not in sys.path:
    sys.path.insert(0, "/opt/trn_rl_repo")

import numpy as np
import ml_dtypes
import concourse.bass as bass
import concourse.bacc as bacc
import concourse.tile as tile
from concourse import mybir
from concourse.bass_utils import run_bass_kernel_spmd
from contextlib import ExitStack

F32 = mybir.dt.float32
F8E4 = mybir.dt.float8e4
BF16 = mybir.dt.bfloat16
U8 = mybir.dt.uint8
AFT = mybir.ActivationFunctionType
ALU = mybir.AluOpType
DROW = mybir.MatmulPerfMode.DoubleRow

T, BF, D, H, K, NC = 128, 256, 256, 512, 4, 8
BL = BF // NC          # 32 batch per core
G = 4 * H              # 2048
NEG_INF = -1e9

_cached = {}


def _build_program(t_steps=T):
    nc = bacc.Bacc()
    # ---- DRAM I/O ----
    d_xT = nc.declare_dram_parameter("xT", [D, t_steps * BL], BF16, isOutput=False)
    d_wih = nc.declare_dram_parameter("wih", [D, G], BF16, isOutput=False)
    # W_hh^T in fp8e4, DoubleRow pair layout [p, pair, ksub, n] flattened
    d_whh8 = nc.declare_dram_parameter("whh8", [128, 2 * 2 * G], U8, isOutput=False)
    d_biasrow = nc.declare_dram_parameter("biasrow", [1, G], BF16, isOutput=False)
    d_ones = nc.declare_dram_parameter("onesrow", [1, 128], BF16, isOutput=False)
    d_u1t = nc.declare_dram_parameter("u1t", [128, 4], BF16, isOutput=False)
    d_u2t = nc.declare_dram_parameter("u2t", [128, 8], BF16, isOutput=False)
    d_i32s = nc.declare_dram_parameter("i32s", [128, 32], BF16, isOutput=False)
    d_i128 = nc.declare_dram_parameter("i128", [128, 128], BF16, isOutput=False)
    d_maskneg = nc.declare_dram_parameter("maskneg", [BL, K * t_steps], F32, isOutput=False)
    d_valid = nc.declare_dram_parameter("valid", [BL, K], F32, isOutput=False)
    d_out = nc.declare_dram_parameter("out", [BL * K, H], F32, isOutput=True)

    NRC = t_steps // 4     # xw row chunks of 128 rows (4 timesteps each)

    with tile.TileContext(nc) as tc, ExitStack() as ctx:
        nv, ns, nt = nc.vector, nc.scalar, nc.tensor

        consts = ctx.enter_context(tc.tile_pool(name="consts", bufs=1))
        big = ctx.enter_context(tc.tile_pool(name="big", bufs=1))

        # ---- load constants ----
        wih_sb = [consts.tile([128, G], BF16, tag=f"wih{i}", name=f"wih{i}")
                  for i in range(2)]
        for i in range(2):
            nc.sync.dma_start(wih_sb[i][:], d_wih[128 * i:128 * (i + 1), :])
        whh8_sb = consts.tile([128, 2 * 2 * G], F8E4, tag="whh8")
        nc.sync.dma_start(whh8_sb[:], d_whh8[:].bitcast(F8E4))
        whh8_r = whh8_sb[:].rearrange("p (pr ks n) -> p pr ks n", pr=2, ks=2)
        biasrow_sb = consts.tile([1, G], BF16, tag="biasrow")
        nc.sync.dma_start(biasrow_sb[:], d_biasrow[:])
        ones_sb = consts.tile([1, 128], BF16, tag="ones")
        nc.sync.dma_start(ones_sb[:], d_ones[:])
        u1t_sb = consts.tile([128, 4], BF16, tag="u1t")
        nc.sync.dma_start(u1t_sb[:], d_u1t[:])
        u2t_sb = consts.tile([128, 8], BF16, tag="u2t")
        nc.sync.dma_start(u2t_sb[:], d_u2t[:])
        i32s_bf = consts.tile([128, 32], BF16, tag="i32s_bf")
        nc.sync.dma_start(i32s_bf[:], d_i32s[:])
        i128_bf = consts.tile([128, 128], BF16, tag="i128_bf")
        nc.sync.dma_start(i128_bf[:], d_i128[:])
        maskneg_sb = consts.tile([BL, K * t_steps], F32, tag="maskneg")
        nc.sync.dma_start(maskneg_sb[:], d_maskneg[:])
        valid_sb = consts.tile([BL, K], F32, tag="valid")
        nc.sync.dma_start(valid_sb[:], d_valid[:])

        # ---- persistent state ----
        xw_all = big.tile([128, NRC * G], BF16, tag="xw_all")      # 128KB/part
        hsT = big.tile([128, t_steps * 128], BF16, tag="hsT")      # [p, t*128+c*32+b]
        c_sb = big.tile([BL, H], BF16, tag="c")
        a1ch = big.tile([128, NRC], F32, tag="a1ch")               # a1 by row-chunk
        a2_sb = big.tile([BL, t_steps], F32, tag="a2")

        # ---- preamble: xw for the whole sequence, one continuous stream ----
        pre_ctx = ExitStack()
        xt_pool = pre_ctx.enter_context(tc.tile_pool(name="xt", bufs=4))
        ps_pre = pre_ctx.enter_context(tc.tile_pool(name="ps_pre", bufs=2, space="PSUM"))
        ps_pa1 = pre_ctx.enter_context(tc.tile_pool(name="ps_pa1", bufs=2, space="PSUM"))
        for rr in range(NRC):
            xtc = [xt_pool.tile([128, 128], BF16, tag=f"xtc{kd}",
                                name=f"xtc{kd}_{rr}") for kd in range(2)]
            for kd in range(2):
                nc.sync.dma_start(
                    xtc[kd][:],
                    d_xT[128 * kd:128 * (kd + 1), 128 * rr:128 * (rr + 1)])
            for half in range(2):
                pxw = ps_pre.tile([128, 1024], F32, tag="pxw")
                for n2 in range(2):
                    col = 1024 * half + 512 * n2
                    for kd in range(2):
                        nt.matmul(pxw[:, 512 * n2:512 * (n2 + 1)], xtc[kd],
                                  wih_sb[kd][:, col:col + 512],
                                  start=(kd == 0), stop=False)
                    nt.matmul(pxw[:, 512 * n2:512 * (n2 + 1)], ones_sb[:],
                              biasrow_sb[:, col:col + 512],
                              start=False, stop=True)
                dst = xw_all[:, rr * G + 1024 * half:rr * G + 1024 * (half + 1)]
                if half == 0:
                    ns.copy(dst, pxw[:])
                else:
                    nv.tensor_copy(dst, pxw[:])
            pa1 = ps_pa1.tile([128, 2], F32, tag="pa1")
            for kd in range(2):
                nt.matmul(pa1[:], xtc[kd], u1t_sb[:, 2 * kd:2 * kd + 2],
                          start=(kd == 0), stop=(kd == 1))
            ns.copy(a1ch[:, rr:rr + 1], pa1[:, 0:1])
        pre_ctx.close()

        # ---- loop pools ----
        loop_ctx = ExitStack()
        gate_pool = loop_ctx.enter_context(tc.tile_pool(name="gate", bufs=2))
        tmp_pool = loop_ctx.enter_context(tc.tile_pool(name="tmp", bufs=2))
        tct_pool = loop_ctx.enter_context(tc.tile_pool(name="tct", bufs=2))
        h8_pool = loop_ctx.enter_context(tc.tile_pool(name="h8", bufs=2))
        ps_z = loop_ctx.enter_context(tc.tile_pool(name="ps_z", bufs=1, space="PSUM"))
        ps_oc = loop_ctx.enter_context(tc.tile_pool(name="ps_oc", bufs=1, space="PSUM"))

        pz_tiles = {}

        def emit_injects(t):
            """Identity matmuls seeding pz[t] with xw rows (+bias).

            One PSUM tile PER GATE so each bank's accumulation group closes
            independently — readers (per-gate activations) otherwise wait
            for the whole tile's group, serializing the tail after all whh.
            """
            pzs = [ps_z.tile([BL, 512], F32, tag=f"pz{n}", name=f"pz{n}_{t}")
                   for n in range(4)]
            pz_tiles[t] = pzs
            rn, tn4 = divmod(t, 4)
            last = (t == 0)   # t=0 has no whh accumulation
            for n in range(4):
                nt.matmul(pzs[n][:],
                          i32s_bf[32 * tn4:32 * (tn4 + 1), :],
                          xw_all[32 * tn4:32 * (tn4 + 1),
                                 rn * G + 512 * n:rn * G + 512 * (n + 1)],
                          start=True, stop=last,
                          tile_position=(32 * tn4, 0))

        emit_injects(0)

        h8_prev = None
        # gate column order: n0=g, n1=f, n2=i, n3=o (host perm matches)
        for t in range(t_steps):
            pzs = pz_tiles.pop(t)
            # --- PE: whh accumulation, fp8 DoubleRow (K=256 per pass),
            # n-outer so gate chunks finish early
            if t > 0:
                h8p = h8_prev[:].rearrange("p (pr ks b) -> p pr ks b",
                                           pr=2, ks=2)
                for n in range(4):
                    for pr in range(2):
                        nt.matmul(
                            pzs[n][:], h8p[:, pr],
                            whh8_r[:, pr, :, 512 * n:512 * (n + 1)],
                            start=False, stop=(pr == 1), perf_mode=DROW)
            # --- ACT: per-gate activations (z cols [g | f | i | o]) —
            # f right after g so the c-chain (tfc) starts earliest
            gg = gate_pool.tile([BL, 512], BF16, tag="gg")
            ns.activation(gg[:], pzs[0][:], AFT.Tanh)
            sf = gate_pool.tile([BL, 512], BF16, tag="sf")
            ns.activation(sf[:], pzs[1][:], AFT.Sigmoid)
            si = gate_pool.tile([BL, 512], BF16, tag="si")
            ns.activation(si[:], pzs[2][:], AFT.Sigmoid)
            so = gate_pool.tile([BL, 512], BF16, tag="so")
            ns.activation(so[:], pzs[3][:], AFT.Sigmoid)
            # --- DVE: c update (tfc first — it only needs sf and c)
            if t == 0:
                nv.tensor_tensor(c_sb[:], si[:], gg[:], op=ALU.mult)
            else:
                tfc = tmp_pool.tile([BL, H], BF16, tag="tfc")
                nv.tensor_tensor(tfc[:], sf[:], c_sb[:], op=ALU.mult)
                tig = tmp_pool.tile([BL, H], BF16, tag="tig")
                nv.tensor_tensor(tig[:], si[:], gg[:], op=ALU.mult)
                nv.tensor_tensor(c_sb[:], tfc[:], tig[:], op=ALU.add)
            # --- PE: transposes of sig_o and c into one PSUM tile
            psOC = ps_oc.tile([128, 256], BF16, tag="psOC")
            for c4 in range(4):
                nt.transpose(psOC[:, 32 * c4:32 * (c4 + 1)],
                             so[:, 128 * c4:128 * (c4 + 1)], i32s_bf[0:32, :])
            for c4 in range(4):
                nt.transpose(psOC[:, 128 + 32 * c4:128 + 32 * (c4 + 1)],
                             c_sb[:, 128 * c4:128 * (c4 + 1)], i32s_bf[0:32, :])
            # --- PE: injects for t+1 (fill the tail stall, keep PE warm)
            if t + 1 < t_steps:
                emit_injects(t + 1)
            # --- ACT: tanh on transposed c (128-partition, short free dim)
            tcT = tct_pool.tile([128, 128], BF16, tag="tcT")
            ns.activation(tcT[:], psOC[:, 128:256], AFT.Tanh)
            # --- DVE: h8 = fp8(sig_oT * tanh_cT) feeds next whh first, then
            # the bf16 hsT history write (off the critical path)
            h8 = h8_pool.tile([128, 128], F8E4, tag="h8")
            nv.tensor_tensor(h8[:], psOC[:, 0:128], tcT[:], op=ALU.mult)
            h8_prev = h8
            nv.tensor_tensor(hsT[:, t * 128:(t + 1) * 128],
                             psOC[:, 0:128], tcT[:], op=ALU.mult)

        loop_ctx.close()

        # ---- post-loop: batched a2 = u2 . h_t over all (b, t) ----
        a2ctx = ExitStack()
        ps_a2b = a2ctx.enter_context(tc.tile_pool(name="ps_a2b", bufs=8, space="PSUM"))
        a2stg = a2ctx.enter_context(tc.tile_pool(name="a2stg", bufs=8))
        hsT_bt = hsT[:].rearrange("p (t c b) -> p c b t", c=4, b=BL)
        for j in range(8):
            pa2b = ps_a2b.tile([1, 512], F32, tag="pa2b")
            for c4 in range(4):
                nt.matmul(pa2b[:], u2t_sb[:, 2 * c4:2 * c4 + 1],
                          hsT_bt[:, c4, 4 * j:4 * (j + 1), :],
                          start=(c4 == 0), stop=(c4 == 3))
            a2r = a2stg.tile([1, 512], F32, tag="a2r")
            if j % 2 == 0:
                ns.copy(a2r[:], pa2b[:])
            else:
                nv.tensor_copy(a2r[:], pa2b[:])
            for bb in range(4):
                nc.sync.dma_start(a2_sb[4 * j + bb:4 * j + bb + 1, :],
                                  a2r[0:1, 128 * bb:128 * (bb + 1)])
        a2ctx.close()

        # ---- post-loop: attention scores + softmax + pooling ----
        post = ctx.enter_context(tc.tile_pool(name="post", bufs=1))
        ps_t = ctx.enter_context(tc.tile_pool(name="ps_t", bufs=2, space="PSUM"))
        ps_pool = ctx.enter_context(tc.tile_pool(name="ps_pool", bufs=4, space="PSUM"))
        stg_pool = ctx.enter_context(tc.tile_pool(name="stg", bufs=4))
        hsb_pool = ctx.enter_context(tc.tile_pool(name="hsb", bufs=2))

        # a1 assembly: a1bp[b, 4r+c] = a1ch[32c+b, r]
        a1bp = post.tile([BL, t_steps], F32, tag="a1bp")
        for c in range(4):
            nv.tensor_copy(a1bp[:].rearrange("b (r c) -> b r c", c=4)[:, :, c],
                           a1ch[32 * c:32 * (c + 1), :])
        abp = post.tile([BL, t_steps], F32, tag="abp")
        nv.tensor_tensor(abp[:], a1bp[:], a2_sb[:], op=ALU.add)
        ns.activation(abp[:], abp[:], AFT.Sigmoid)

        # softmax per slot k -> wT [t, 4b+k] (bf16 for the pooling matmul)
        wT = post.tile([t_steps, K * BL], BF16, tag="wT")
        for k in range(K):
            sc = post.tile([BL, t_steps], F32, tag=f"sc{k}")
            nv.tensor_tensor(sc[:], abp[:],
                             maskneg_sb[:, t_steps * k:t_steps * (k + 1)], op=ALU.add)
            mneg = post.tile([BL, 1], F32, tag=f"mneg{k}")
            nv.tensor_reduce(mneg[:], sc[:], axis=mybir.AxisListType.X,
                             op=ALU.max, negate=True)
            ek = post.tile([BL, t_steps], F32, tag=f"ek{k}")
            sk = post.tile([BL, 1], F32, tag=f"sk{k}")
            ns.activation(ek[:], sc[:], AFT.Exp, bias=mneg[:], accum_out=sk[:])
            rk = post.tile([BL, 1], F32, tag=f"rk{k}")
            nv.reciprocal(rk[:], sk[:])
            wk = post.tile([BL, t_steps], BF16, tag=f"wk{k}")
            nv.tensor_scalar(out=wk[:], in0=ek[:], scalar1=rk[:],
                             scalar2=valid_sb[:, k:k + 1], op0=ALU.mult, op1=ALU.mult)
            # transpose into wT columns k::4  (wT[t, 4b+k])
            pwT = ps_t.tile([128, 32], BF16, tag="pwT")
            nt.transpose(pwT[0:t_steps, :], wk[:], i32s_bf[0:32, :])
            nv.tensor_copy(wT[:].rearrange("t (b k) -> t b k", k=4)[:, :, k],
                           pwT[0:t_steps, :])

        # pooling: per b, rebuild hs_b [t, h] via 4 PE transposes, then [4,T]@[T,H]
        hsT_r = hsT[:].rearrange("p (t c b) -> p t c b", c=4, b=BL)
        for b in range(BL):
            hsb = hsb_pool.tile([t_steps, H], BF16, tag="hsb")
            for c in range(4):
                pt = ps_t.tile([128, 128], BF16, tag="pt")
                nt.transpose(pt[0:t_steps, :], hsT_r[:, :, c, b], i128_bf[:])
                if c % 2 == 0:
                    ns.copy(hsb[:, 128 * c:128 * (c + 1)], pt[0:t_steps, :])
                else:
                    nv.tensor_copy(hsb[:, 128 * c:128 * (c + 1)], pt[0:t_steps, :])
            pp = ps_pool.tile([K, H], F32, tag="pp")
            nt.matmul(pp[:], wT[0:t_steps, 4 * b:4 * (b + 1)], hsb[:],
                      start=True, stop=True)
            so = stg_pool.tile([K, H], F32, tag="so")
            ns.copy(so[:], pp[:])
            nc.sync.dma_start(d_out[K * b:K * (b + 1), :], so[:])

    nc.compile()
    return nc


def _host_prep(x, W_ih, W_hh, b_ih, b_hh, A1, A2, v1, lengths, label_len):
    assert int(label_len) == K
    BFD = ml_dtypes.bfloat16
    # torch gate rows (i,f,g,o) -> z column order (g,f,i,o)
    perm = np.concatenate([np.arange(1024, 1536), np.arange(512, 1024),
                           np.arange(0, 512), np.arange(1536, 2048)])
    wih = np.ascontiguousarray(W_ih[perm].T).astype(BFD)                # [256, 2048]
    whhT = np.asarray(W_hh[perm].T, dtype=np.float32)                   # [512, 2048]
    # fp8 DoubleRow pair layout: whh8[p, pr, ks, n] = WhhT[256*pr+128*ks+p, n]
    whh8 = whhT.reshape(2, 2, 128, G).transpose(2, 0, 1, 3).reshape(128, 4 * G)
    whh8 = np.ascontiguousarray(whh8).astype(ml_dtypes.float8_e4m3).view(np.uint8)
    biasrow = ((b_ih + b_hh)[perm]).astype(np.float32).reshape(1, G).astype(BFD)
    u1 = (v1 @ A1)[0].astype(np.float32)                                # [256]
    u2 = (v1 @ A2)[0].astype(np.float32)                                # [512]
    u1t = np.zeros((128, 4), dtype=np.float32)                          # [128, 4]
    u1t[:, 0] = u1[0:128]
    u1t[:, 2] = u1[128:256]
    u2t = np.zeros((128, 8), dtype=np.float32)                          # [128, 8]
    for c in range(4):
        u2t[:, 2 * c] = u2[128 * c:128 * (c + 1)]
    i32s = np.zeros((128, 32), dtype=np.float32)
    i32s[np.arange(128), np.arange(128) % 32] = 1.0
    i128 = np.eye(128, dtype=np.float32)

    shared = dict(wih=wih, whh8=whh8, biasrow=biasrow,
                  u1t=u1t.astype(BFD), u2t=u2t.astype(BFD),
                  i32s=i32s.astype(BFD), i128=i128.astype(BFD),
                  onesrow=np.ones((1, 128), dtype=np.float32).astype(BFD))

    in_maps = []
    for cidx in range(NC):
        sl = slice(cidx * BL, (cidx + 1) * BL)
        xc = x[:, sl, :]                                                # [T, 32, D]
        xT = np.ascontiguousarray(xc.reshape(T * BL, D).T).astype(BFD)
        ln = lengths[sl].astype(np.int64)
        t_start = np.maximum(ln - K, 0)
        t_k = t_start[:, None] + np.arange(K)[None, :]                  # [32, 4]
        valid = (t_k <= (ln[:, None] - 1))                              # [32, 4]
        tt = np.arange(T)
        mask = (tt[None, None, :] <= t_k[:, :, None]) & valid[:, :, None]  # [b, k, t]
        maskneg = np.where(mask, 0.0, NEG_INF).astype(np.float32)
        maskneg = np.ascontiguousarray(maskneg.reshape(BL, K * T))      # k-major cols
        in_maps.append(dict(shared, xT=xT, maskneg=maskneg,
                            valid=valid.astype(np.float32)))
    return in_maps


def kernel(**inputs) -> np.ndarray:
    inputs = {k: np.asarray(v) if not np.isscalar(v) else v for k, v in inputs.items()}
    in_maps = _host_prep(**inputs)
    if "nc" not in _cached:
        _cached["nc"] = _build_program()
    nc = _cached["nc"]
    res = run_bass_kernel_spmd(nc, in_maps, core_ids=list(range(NC)))
    outs = []
    for cidx in range(NC):
        o = res.results[cidx]["out"]                                    # [128, 512]
        outs.append(o.reshape(BL, K, H))
    return np.concatenate(outs, axis=0).astype(np.float32)              # [256, 4, 512]
